# revision 1
# baseline (speedup 1.0000x reference)
"""Trainium2 Bass kernel: ablation-style attention (nn_Attention).

Full inputs -> full output [4, 14, 1024, 768] f32.

Sharding: 8 cores = 4 batches x 2 query-halves. Each core computes both
residual streams, all 12 heads, for its 512 queries against all 1024 keys.
Keys are cyclically rotated per core so queries are always key-columns
0..511, making the causal-mask structure identical on every core (SPMD:
one graph, per-core data). Zero collectives; host does pure concatenation.

Raw bass (explicit semaphores): this toolchain's walrus build only encodes
a single sync-wait per instruction, so TileContext output (multi-wait
instructions) cannot be lowered; standalone wait_ge instructions are used
instead.

Math per core:
  q_T/k_T [nh=768, s] and v [s, nh-augmented] from bf16 matmuls of x^T.
  Per head: S_T [sk, sq] = k_h^T.T @ q_h^T ; P = exp(S/8) (scores are tiny:
  no max subtraction; masking = post-exp multiply for diagonal key-slots,
  exp-bias -30000 for all-masked slots). z_aug [65, sq] = v_aug^T @ P with
  an appended ones-column in v giving the softmax denominator in row 64.
  Normalize via reciprocal + K=1 broadcast matmul.
  ch0 = z0@Wo + b_O, ch1 = z1@Wo + b_O, ch(2+k) = ch1 + (z0_k - z1_k)@Wo_k.
"""

import os
import numpy as np
import ml_dtypes

N_HEADS = 12
D_MODEL = 768
D_HEAD = 64
B = 4
S = 1024
SQ = 512
NT_D = 6     # 768 / 128
NT_SK = 8    # 1024 / 128
NT_SQ = 4    # 512 / 128
VW = 65 * N_HEADS  # 780: per-head 64 v cols + 1 ones col
NPT = 4      # P-tile pair-buffer rotation depth
NCH = 4      # output staging rotation depth
BF16 = ml_dtypes.bfloat16

LAST_EXEC_NS = None
_GRAPH = None


def _build_graph():
    import concourse.bass as bass
    import concourse.mybir as mybir
    from contextlib import ExitStack

    f32 = mybir.dt.float32
    bf16 = mybir.dt.bfloat16
    Exp = mybir.ActivationFunctionType.Exp
    Ident = mybir.ActivationFunctionType.Identity

    nc = bass.Bass()

    xt_d = nc.declare_dram_parameter("xt", [128, 2, NT_D, S], bf16, isOutput=False)
    wq_d = nc.declare_dram_parameter("wq", [128, NT_D, 768], bf16, isOutput=False)
    wk_d = nc.declare_dram_parameter("wk", [128, NT_D, 768], bf16, isOutput=False)
    wv_d = nc.declare_dram_parameter("wv", [128, NT_D, VW], bf16, isOutput=False)
    wo_d = nc.declare_dram_parameter("wo", [128, NT_D, 768], bf16, isOutput=False)
    bq_d = nc.declare_dram_parameter("bq", [128, NT_D], f32, isOutput=False)
    bk_d = nc.declare_dram_parameter("bk", [128, NT_D], f32, isOutput=False)
    vb_d = nc.declare_dram_parameter("vb", [1, VW], bf16, isOutput=False)
    bo_d = nc.declare_dram_parameter("bo", [1, 768], bf16, isOutput=False)
    mask_d = nc.declare_dram_parameter("mask", [128, 4, 2 * SQ], bf16, isOutput=False)
    eb_d = nc.declare_dram_parameter("ebias", [128, 4], f32, isOutput=False)
    ind_d = nc.declare_dram_parameter("ind", [12, 768], bf16, isOutput=False)
    id_d = nc.declare_dram_parameter("ident", [128, 128], bf16, isOutput=False)
    out_d = nc.declare_dram_parameter("out", [14, SQ, 768], f32, isOutput=True)

    ctx = ExitStack()
    sb = lambda name, shape, dt: ctx.enter_context(nc.sbuf_tensor(name, shape, dt))
    psa = lambda name, shape: ctx.enter_context(nc.psum_tensor(name, shape, f32))

    xt = sb("xt_s", [128, 2, NT_D, S], bf16)
    wq = sb("wq_s", [128, NT_D, 768], bf16)
    wk = sb("wk_s", [128, NT_D, 768], bf16)
    wv = sb("wv_s", [128, NT_D, VW], bf16)
    wo = sb("wo_s", [128, NT_D, 768], bf16)
    bq = sb("bq_s", [128, NT_D], f32)
    bk = sb("bk_s", [128, NT_D], f32)
    vb = sb("vb_s", [1, VW], bf16)
    bo = sb("bo_s", [1, 768], bf16)
    maskt = sb("mask_s", [128, 4, 2 * SQ], bf16)
    ebt = sb("ebt_s", [128, 4], f32)
    ind = sb("ind_s", [12, 768], bf16)
    idn = sb("ident_s", [128, 128], bf16)
    ones_b = sb("ones_b", [1, S], bf16)

    qT = sb("qT", [128, 2, NT_D, SQ], bf16)
    kT = sb("kT", [128, 2, NT_D, S], bf16)
    vA = sb("vA", [128, 2, NT_SK, VW], bf16)
    zT = sb("zT", [128, 2, NT_D, SQ], bf16)
    dzt = sb("dzt", [128, NT_D, SQ], bf16)
    Ssb0 = sb("Ssb0", [128, NT_SQ, 768], f32)
    Ssb16 = sb("Ssb16", [128, NT_SQ, 768], bf16)
    pts = [sb(f"pt{i}", [128, 2 * SQ], bf16) for i in range(NPT)]
    den_s = sb("den_s", [1, 4 * SQ], f32)
    den12 = sb("den12", [64, SQ], f32)
    recip_s = sb("recip_s", [12, SQ], bf16)
    chb = [sb(f"chb{i}", [128, 768], f32) for i in range(NCH)]

    psALL = psa("psALL", [128, 4 * 512])   # 4 rotating bank slots
    psZ = [psa(f"psZ{i}", [65, SQ]) for i in range(2)]
    psB = [psa(f"psB{i}", [128, SQ]) for i in range(2)]

    class Ctr:
        __slots__ = ("sem", "n")

        def __init__(self, name):
            self.sem = ctx.enter_context(nc.semaphore(name))
            self.n = 0

    G = [Ctr(f"g{i}") for i in range(6)]
    PEc = Ctr("pe")
    ACTc = Ctr("act")
    DVEc = Ctr("dve")
    CH = [Ctr(f"ch{i}") for i in range(8)]
    DN = Ctr("dn")

    prog = {k: [] for k in ("pe", "act", "dve", "sync")}
    observed = {k: {} for k in prog}

    def op(eng, fn):
        prog[eng].append(fn)

    def wait(eng, ctr, val):
        if val is None or val <= 0:
            return
        key = id(ctr)
        if observed[eng].get(key, 0) >= val:
            return
        observed[eng][key] = val
        op(eng, lambda e, s=ctr.sem, v=val: e.wait_ge(s, v))

    def emit(eng, build, inc=None, k=1):
        ev = None
        if inc is not None:
            inc.n += k
            ev = inc.n

        def f(e, b=build, i=inc, kk=k):
            r = b(e)
            if i is not None:
                r.then_inc(i.sem, kk)

        op(eng, f)
        return ev

    # ---------------- DVE constants ----------------
    ev_ones = emit("dve", lambda e: e.memset(ones_b[:], 1.0), inc=DVEc)
    # warm the ACT exp table during the initial DMA window (the first real
    # exp otherwise pays the ~2.7us ACT_TABLE_LOAD on pair-0's critical path)
    wait("act", DVEc, ev_ones)
    emit("act", lambda e: e.activation(
        den_s[0:1, 0:1], ones_b[0:1, 0:1],
        __import__("concourse.mybir", fromlist=["m"]).ActivationFunctionType.Exp,
        bias=0.0, scale=1.0), inc=ACTc)

    # ---------------- input DMAs (priority order, grouped sems) -------------
    loads = [
        (xt[:, 0], xt_d[:, 0], 0), (wq[:], wq_d[:], 0), (bq[:], bq_d[:], 0),
        (wk[:], wk_d[:], 1), (bk[:], bk_d[:], 1),
        (wv[:], wv_d[:], 2), (vb[:], vb_d[:], 2),
        (xt[:, 1], xt_d[:, 1], 3),
        (maskt[:], mask_d[:], 4), (ebt[:], eb_d[:], 4),
        (wo[:], wo_d[:], 5), (bo[:], bo_d[:], 5), (ind[:], ind_d[:], 5),
        (idn[:], id_d[:], 5),
    ]
    gtot = [0] * 6
    for a_, b_, gi in loads:
        gtot[gi] += 16
    issued = 0
    for a_, b_, gi in loads:
        if issued == 5:
            # give the critical first groups exclusive DMA bandwidth
            wait("sync", G[0], gtot[0])
        emit("sync", lambda e, a=a_, b=b_: e.dma_start(out=a, in_=b),
             inc=G[gi], k=16)
        issued += 1

    # psum slot rotation over psALL's 4 bank slots (each 512 f32 columns).
    # All allocation groups use an even slot count, so pairs stay aligned.
    slot_state = [None] * 4
    slot_i = [0]

    def next_slot():
        idx = slot_i[0] % 4
        slot_i[0] += 1
        war = slot_state[idx]
        if war is not None:
            for ctr_ev in (war if isinstance(war, list) else [war]):
                wait("pe", ctr_ev[0], ctr_ev[1])
        return idx, idx * 512

    # ================= Phase A: QKV projections =================
    wait("pe", DVEc, ev_ones)
    wait("pe", G[0], gtot[0])
    qk_last_ev = [0, 0]
    v_last_ev = [0, 0]

    def emit_q(p):
        for rt in range(NT_D):          # q_T tiles
            idx, off = next_slot()
            for dt in range(NT_D):
                ev = emit("pe", lambda e, o=psALL[:, off:off + SQ],
                          l=wq[:, dt, rt * 128:(rt + 1) * 128],
                          r=xt[:, p, dt, 0:SQ], s=(dt == 0),
                          st_=(dt == NT_D - 1):
                          e.matmul(o, l, r, start=s, stop=st_),
                          inc=PEc if dt == NT_D - 1 else None)
            wait("act", PEc, ev)
            cev = emit("act", lambda e, o=qT[:, p, rt, :],
                       i=psALL[:, off:off + SQ], bb=bq[:, rt:rt + 1]:
                       e.activation(o, i, Ident, bias=bb), inc=ACTc)
            slot_state[idx] = (ACTc, cev)
            qk_last_ev[p] = cev

    def emit_ktile(p, rt):
        if True:
            for half in range(2):
                idx, off = next_slot()
                for dt in range(NT_D):
                    ev = emit("pe", lambda e, o=psALL[:, off:off + 512],
                              l=wk[:, dt, rt * 128:(rt + 1) * 128],
                              r=xt[:, p, dt, half * 512:(half + 1) * 512],
                              s=(dt == 0), st_=(dt == NT_D - 1):
                              e.matmul(o, l, r, start=s, stop=st_),
                              inc=PEc if dt == NT_D - 1 else None)
                wait("act", PEc, ev)
                cev = emit("act", lambda e,
                           o=kT[:, p, rt, half * 512:(half + 1) * 512],
                           i=psALL[:, off:off + 512], bb=bk[:, rt:rt + 1]:
                           e.activation(o, i, Ident, bias=bb), inc=ACTc)
                slot_state[idx] = (ACTc, cev)
                qk_last_ev[p] = cev

    def emit_vtile(p, st):
        # one sk-tile of v_aug: two adjacent slots (nsl 0/1), one batched copy
        idx0, off0 = next_slot()
        idx1, off1 = next_slot()
        for nsl, off in ((0, off0), (1, off1)):
            for dt in range(NT_D):
                emit("pe", lambda e, o=psALL[:, off:off + 390],
                     l=xt[:, p, dt, st * 128:(st + 1) * 128],
                     r=wv[:, dt, nsl * 390:(nsl + 1) * 390], s=(dt == 0):
                     e.matmul(o, l, r, start=s, stop=False))
            inc = PEc if nsl == 1 else None
            ev = emit("pe", lambda e, o=psALL[:, off:off + 390],
                      l=ones_b[0:1, 0:128], r=vb[0:1, nsl * 390:(nsl + 1) * 390]:
                      e.matmul(o, l, r, start=False, stop=True), inc=inc)
        wait("dve", PEc, ev)
        cev = emit("dve", lambda e,
                   o=vA[:, p, st, :].rearrange("p (n f) -> p n f", n=2),
                   i=psALL[:, off0:off0 + 1024].rearrange(
                       "p (n f) -> p n f", n=2)[:, :, 0:390]:
                   e.tensor_copy(o, i), inc=DVEc)
        slot_state[idx0] = (DVEc, cev)
        slot_state[idx1] = (DVEc, cev)
        v_last_ev[p] = cev

    emit_q(0)
    wait("pe", G[1], gtot[1])
    for rt in range(NT_D):
        emit_ktile(0, rt)
    wait("pe", G[2], gtot[2])
    for st in range(NT_SK):
        emit_vtile(0, st)
    wait("pe", G[3], gtot[3])
    emit_q(1)

    # ================= Phase B: attention =================
    wait("act", G[4], gtot[4])
    wait("dve", G[4], gtot[4])
    ssb_ev = {}
    pt_i = [0]
    den_war = [0, 0]
    zrel_prev = [0, 0]
    psb_prev = [0, 0]
    recip_rel = [0]
    z_norm_last = [0, 0]
    zcopy_last = [0, 0]

    def emit_sproj_tile(p, mt, nsl):
        cpc = DVEc if p == 0 else ACTc
        idx, off = next_slot()
        wait("pe", DVEc, z_norm_last[p])
        for kt in range(NT_D):
            emit("pe", lambda e, o=psALL[:, off:off + 384],
                 l=zT[:, p, kt, mt * 128:(mt + 1) * 128],
                 r=wo[:, kt, nsl * 384:(nsl + 1) * 384], s=(kt == 0):
                 e.matmul(o, l, r, start=s, stop=False))
        ev = emit("pe", lambda e, o=psALL[:, off:off + 384],
                  l=ones_b[0:1, 0:128], r=bo[0:1, nsl * 384:(nsl + 1) * 384]:
                  e.matmul(o, l, r, start=False, stop=True), inc=PEc)
        if p == 0:
            wait("dve", PEc, ev)
            cev = emit("dve", lambda e,
                       o=Ssb0[:, mt, nsl * 384:(nsl + 1) * 384],
                       i=psALL[:, off:off + 384]: e.tensor_copy(o, i), inc=DVEc)
            slot_state[idx] = (DVEc, cev)
            if nsl == 1:
                wait("sync", DVEc, cev)
                emit("sync", lambda e, o=out_d[0, mt * 128:(mt + 1) * 128, :],
                     i=Ssb0[:, mt, :]: e.dma_start(out=o, in_=i),
                     inc=CH[mt], k=16)
        else:
            # stream 1: DVE extracts psum -> chb f32; ACT casts chb -> bf16
            # (single psum reader per bank: ScalarE+VectorE may not read the
            # same PSUM bank concurrently)
            c = mt % NCH
            wait("dve", PEc, ev)
            if nsl == 0:
                wait("dve", CH[c], CH[c].n)
            fev = emit("dve", lambda e, o=chb[c][:, nsl * 384:(nsl + 1) * 384],
                       i=psALL[:, off:off + 384]: e.tensor_copy(o, i), inc=DVEc)
            slot_state[idx] = (DVEc, fev)
            wait("act", DVEc, fev)
            cev = emit("act", lambda e,
                       o=Ssb16[:, mt, nsl * 384:(nsl + 1) * 384],
                       i=chb[c][:, nsl * 384:(nsl + 1) * 384]: e.copy(o, i),
                       inc=ACTc)
            ssb_ev[(mt, nsl)] = cev
            if nsl == 1:
                wait("sync", DVEc, fev)
                wait("sync", ACTc, cev)
                emit("sync", lambda e, o=out_d[1, mt * 128:(mt + 1) * 128, :],
                     i=chb[c][:, :]: e.dma_start(out=o, in_=i), inc=CH[c], k=16)

    dn_after = [0, 0]

    def emit_norm(p):
        wait("dve", DN, dn_after[p])
        wait("dve", PEc, recip_rel[0])

        def _recip(e, o=recip_s[:, :], i=den12[32 * p:32 * p + 12, :]):
            with nc.allow_low_precision(reason="softmax denom recip bf16"):
                return e.reciprocal(o, i)

        rev = emit("dve", _recip, inc=DVEc)
        wait("dve", DVEc, zcopy_last[p])
        wait("pe", G[5], gtot[5])
        wait("pe", DVEc, rev)
        for t in range(NT_D):
            bsl = t % 2
            wait("pe", DVEc, psb_prev[bsl])
            bev = emit("pe", lambda e, o=psB[bsl][:, :],
                       l=ind[:, t * 128:(t + 1) * 128], r=recip_s[:, :]:
                       e.matmul(o, l, r, start=True, stop=True), inc=PEc)
            if t == NT_D - 1:
                recip_rel[0] = bev
            wait("dve", PEc, bev)
            mev = emit("dve", lambda e, o=zT[:, p, t, :], b=psB[bsl][:, :]:
                       e.tensor_mul(o, o, b), inc=DVEc)
            psb_prev[bsl] = mev
            z_norm_last[p] = mev

    def attention_stream(p, fillers=None):
        wait("pe", ACTc, qk_last_ev[p])
        wait("pe", DVEc, v_last_ev[p])
        ev_pt = {}
        pt_of = {}
        ev_av_last = {}

        def do_S(g, st):
            heads = (2 * g, 2 * g + 1)
            idx0, off0 = next_slot()
            idx1, off1 = next_slot()
            for h, off in ((heads[0], off0), (heads[1], off1)):
                po = (h % 2) * 64
                inc = PEc if h == heads[1] else None
                ev = emit("pe", lambda e, o=psALL[:, off:off + SQ],
                          l=kT[po:po + 64, p, h // 2, st * 128:(st + 1) * 128],
                          r=qT[po:po + 64, p, h // 2, :]:
                          e.matmul(o, l, r, start=True, stop=True), inc=inc)
            wait("act", PEc, ev)
            u = pt_i[0]
            pt_i[0] += 1
            ptb = pts[u % NPT]
            if st < 4:
                eev = emit("act", lambda e, o=ptb[:],
                           i=psALL[:, off0:off0 + 2 * SQ]:
                           e.activation(o, i, Exp, bias=0.0, scale=0.125),
                           inc=ACTc)
                wait("dve", ACTc, eev)
                mev = emit("dve", lambda e, o=ptb[:], m=maskt[:, st, :]:
                           e.tensor_mul(o, o, m), inc=DVEc)
                ev_pt[(g, st)] = (DVEc, mev)
            else:
                eev = emit("act", lambda e, o=ptb[:],
                           i=psALL[:, off0:off0 + 2 * SQ],
                           b=ebt[:, st - 4:st - 3]:
                           e.activation(o, i, Exp, bias=b, scale=0.125),
                           inc=ACTc)
                ev_pt[(g, st)] = (ACTc, eev)
            slot_state[idx0] = (ACTc, eev)
            slot_state[idx1] = (ACTc, eev)
            pt_of[(g, st)] = ptb

        def do_AV(g, st, h):
            zsl = h % 2
            ctr, v = ev_pt[(g, st)]
            wait("pe", ctr, v)
            if st == 0:
                wait("pe", DVEc, zrel_prev[zsl])
            inc = PEc if st == NT_SK - 1 else None
            ev = emit("pe", lambda e, o=psZ[zsl][0:65, :],
                      l=vA[:, p, st, 65 * h:65 * h + 65],
                      r=pt_of[(g, st)][:, zsl * SQ:(zsl + 1) * SQ]:
                      e.matmul(o, l, r, start=(st == 0),
                               stop=(st == NT_SK - 1)), inc=inc)
            if ev is not None:
                ev_av_last[(g, h)] = ev

        for g in range(6):
            heads = (2 * g, 2 * g + 1)
            do_S(g, 0)
            do_S(g, 1)
            if fillers and g < len(fillers) and fillers[g]:
                fillers[g]()
            for st in range(NT_SK):
                for h in heads:
                    do_AV(g, st, h)
                if st + 2 < NT_SK:
                    do_S(g, st + 2)

            dsl = g % 2
            for h in heads:
                zsl = h % 2
                po = (h % 2) * 64
                wait("dve", PEc, ev_av_last[(g, h)])
                wait("dve", DN, DN.n)
                zcev = emit("dve", lambda e, o=zT[po:po + 64, p, h // 2, :],
                            i=psZ[zsl][0:64, :]: e.tensor_copy(o, i), inc=DVEc)
                zcopy_last[p] = zcev
                dev = emit("dve", lambda e, o=den_s[0:1, (dsl * 2 + h % 2) * SQ:
                           (dsl * 2 + h % 2 + 1) * SQ],
                           i=psZ[zsl][64:65, :]: e.tensor_copy(o, i), inc=DVEc)
                zrel_prev[zsl] = dev
            wait("sync", DVEc, zrel_prev[1])
            emit("sync", lambda e, o=den12[32 * p + 2 * g:32 * p + 2 * g + 2, :],
                 i=den_s[0:1, dsl * 2 * SQ:(dsl * 2 + 2) * SQ]:
                 e.dma_start(out=o, in_=i), inc=DN, k=16)
        dn_after[p] = DN.n

    # stream 0 attention: stream-1 k/v projections fill the PE gaps
    v1_fillers = [
        lambda: (emit_ktile(1, 0), emit_ktile(1, 1), emit_vtile(1, 0)),
        lambda: (emit_ktile(1, 2), emit_ktile(1, 3), emit_vtile(1, 1)),
        lambda: (emit_ktile(1, 4), emit_ktile(1, 5), emit_vtile(1, 2)),
        lambda: (emit_vtile(1, 3), emit_vtile(1, 4)),
        lambda: (emit_vtile(1, 5), emit_vtile(1, 6), emit_vtile(1, 7)),
        None,
    ]
    attention_stream(0, v1_fillers)

    fillers1 = [
        None,
        lambda: emit_norm(0),
        lambda: (emit_sproj_tile(0, 0, 0), emit_sproj_tile(0, 0, 1)),
        lambda: (emit_sproj_tile(0, 1, 0), emit_sproj_tile(0, 1, 1)),
        lambda: (emit_sproj_tile(0, 2, 0), emit_sproj_tile(0, 2, 1)),
        lambda: (emit_sproj_tile(0, 3, 0), emit_sproj_tile(0, 3, 1)),
    ]
    attention_stream(1, fillers1)
    emit_norm(1)

    # ================= Phase C: output projections =================
    dzev = None
    wait("dve", DVEc, z_norm_last[1])
    for t in range(NT_D):
        dzev = emit("dve", lambda e, o=dzt[:, t, :], a=zT[:, 0, t, :],
                    b=zT[:, 1, t, :]: e.tensor_sub(o, a, b),
                    inc=DVEc if t == NT_D - 1 else None)

    for mt in range(NT_SQ):
        for nsl in range(2):
            emit_sproj_tile(1, mt, nsl)

    chidx = [0]
    for h in range(N_HEADS):
        po = (h % 2) * 64
        g = h // 2
        for mt in range(NT_SQ):
            idx0, off0 = next_slot()
            idx1, off1 = next_slot()
            wait("pe", DVEc, dzev)
            for nsl, off in ((0, off0), (1, off1)):
                inc = PEc if nsl == 1 else None
                ev = emit("pe", lambda e, o=psALL[:, off:off + 384],
                          l=dzt[po:po + 64, g, mt * 128:(mt + 1) * 128],
                          r=wo[po:po + 64, g, nsl * 384:(nsl + 1) * 384]:
                          e.matmul(o, l, r, start=True, stop=True), inc=inc)
            c = chidx[0] % NCH
            chidx[0] += 1
            wait("dve", PEc, ev)
            wait("dve", ACTc, ssb_ev[(mt, 1)])
            wait("dve", CH[c], CH[c].n)
            src_ap = psALL[:, off0:off0 + 1024].rearrange(
                "p (n f) -> p n f", n=2)[:, :, 0:384]
            dst_ap = chb[c][:, :].rearrange("p (n f) -> p n f", n=2)
            s1_ap = Ssb16[:, mt, :].rearrange("p (n f) -> p n f", n=2)
            aev = emit("dve", lambda e, o=dst_ap, a=src_ap, b=s1_ap:
                       e.tensor_add(o, a, b), inc=DVEc)
            slot_state[idx0] = (DVEc, aev)
            slot_state[idx1] = (DVEc, aev)
            wait("sync", DVEc, aev)
            emit("sync", lambda e, o=out_d[2 + h, mt * 128:(mt + 1) * 128, :],
                 i=chb[c][:, :]: e.dma_start(out=o, in_=i), inc=CH[c], k=16)

    for c in range(8):
        wait("sync", CH[c], CH[c].n)

    # ---------------- emit per-engine streams ----------------
    with nc.Block() as block:
        @block.tensor
        def _(e):
            for fn in prog["pe"]:
                fn(e)

        @block.scalar
        def _(e):
            for fn in prog["act"]:
                fn(e)

        @block.vector
        def _(e):
            for fn in prog["dve"]:
                fn(e)

        @block.sync
        def _(e):
            for fn in prog["sync"]:
                fn(e)

    ctx.close()
    return nc


def _prep_in_maps(inputs):
    nrp = np.asarray(inputs["normalized_resid_pre"], np.float32)
    alt = np.asarray(inputs["alt_normalized_resid_pre"], np.float32)
    WQ = np.asarray(inputs["W_Q"], np.float32)
    bQ = np.asarray(inputs["b_Q"], np.float32)
    WK = np.asarray(inputs["W_K"], np.float32)
    bK = np.asarray(inputs["b_K"], np.float32)
    WV = np.asarray(inputs["W_V"], np.float32)
    bV = np.asarray(inputs["b_V"], np.float32)
    WO = np.asarray(inputs["W_O"], np.float32)
    bO = np.asarray(inputs["b_O"], np.float32)

    def to_tiles(w):  # [768, C] -> [128, NT_D, C] with rows = (t*128 + p)
        return np.ascontiguousarray(
            w.reshape(NT_D, 128, w.shape[1]).transpose(1, 0, 2)
        )

    wq = to_tiles(WQ.transpose(1, 0, 2).reshape(768, 768)).astype(BF16)
    wk = to_tiles(WK.transpose(1, 0, 2).reshape(768, 768)).astype(BF16)
    wv_aug = np.zeros((768, VW), np.float32)
    vb_row = np.zeros((1, VW), np.float32)
    for h in range(N_HEADS):
        wv_aug[:, 65 * h:65 * h + 64] = WV[h]
        vb_row[0, 65 * h:65 * h + 64] = bV[h]
        vb_row[0, 65 * h + 64] = 1.0
    wv = to_tiles(wv_aug).astype(BF16)
    wo = to_tiles(WO.reshape(768, 768)).astype(BF16)

    bq_r = np.ascontiguousarray(
        bQ.reshape(NT_D, 128).T).astype(np.float32)   # [128, 6]
    bk_r = np.ascontiguousarray(
        bK.reshape(NT_D, 128).T).astype(np.float32)
    jj = np.arange(12)[:, None]
    tt = np.arange(NT_D)[None, :, None]
    rr = np.arange(128)[None, None, :]
    ind = (jj == (2 * tt + (rr >= 64)).reshape(1, 768)).astype(BF16)
    vb_r = vb_row.astype(BF16)
    bo_r = bO.reshape(1, 768).astype(BF16)

    r = np.arange(128)[:, None, None]
    t = np.arange(4)[None, :, None]
    i = np.arange(SQ)[None, None, :]
    mask1 = ((128 * t + r) <= i).astype(np.float32)
    mask = np.concatenate([mask1, mask1], axis=2).astype(BF16)  # [128, 4, 1024]

    in_maps = []
    for b in range(B):
        for j in range(2):
            x0 = nrp[b, 0]
            x1 = alt[b]
            xs = []
            for x in (x0, x1):
                xr = np.concatenate([x[512 * j:], x[:512 * j]], axis=0)  # [1024,768]
                xs.append(np.ascontiguousarray(xr.T))  # [768, 1024]
            xtc = np.stack(xs)  # [2, 768, 1024]
            xt_t = np.ascontiguousarray(
                xtc.reshape(2, NT_D, 128, S).transpose(2, 0, 1, 3)
            ).astype(BF16)  # [128, 2, 6, 1024]
            ebias = np.full((128, 4), (-30000.0 if j == 0 else 0.0), np.float32)
            in_maps.append({
                "xt": xt_t, "wq": wq, "wk": wk, "wv": wv, "wo": wo,
                "bq": bq_r, "bk": bk_r, "vb": vb_r, "bo": bo_r,
                "mask": mask, "ebias": ebias, "ind": ind,
                "ident": np.eye(128, dtype=np.float32).astype(BF16),
            })
    return in_maps


def _ensure_profile_hook():
    """Register the NTFF profile hook if the image's antenv lacks it."""
    import sys
    import types

    try:
        from antenv.axon_hooks import get_axon_ntff_profile_hook  # noqa: F401
        return True
    except ImportError:
        pass
    try:
        from trn_agent_boot.trn_boot import _ntff_profile_via_ctypes

        hook = _ntff_profile_via_ctypes("/opt/axon/libaxon_pjrt.so")
        if hook is None:
            return False
        mod = types.ModuleType("antenv.axon_hooks")
        state = {"hook": hook}
        mod.set_axon_ntff_profile_hook = lambda h: state.update(hook=h)
        mod.get_axon_ntff_profile_hook = lambda: state["hook"]
        sys.modules["antenv.axon_hooks"] = mod
        import antenv

        antenv.axon_hooks = mod
        return True
    except Exception:
        return False


def kernel(**inputs):
    global LAST_EXEC_NS, _GRAPH
    from concourse.bass_utils import run_bass_kernel_spmd

    if _GRAPH is None:
        _GRAPH = _build_graph()
    nc = _GRAPH
    in_maps = _prep_in_maps(inputs)
    trace = os.environ.get("KERNEL_PROFILE", "0") == "1"
    if trace:
        trace = _ensure_profile_hook()
    res = run_bass_kernel_spmd(nc, in_maps, list(range(8)), trace=trace)
    LAST_EXEC_NS = res.exec_time_ns
    out = np.empty((B, 14, S, D_MODEL), np.float32)
    for b in range(B):
        for j in range(2):
            out[b, :, 512 * j:512 * (j + 1), :] = res.results[b * 2 + j]["out"]
    return out

